# revision 1
# baseline (speedup 1.0000x reference)
import math
import numpy as np

D_MODEL = 1024
D_HEAD = 64
N_HEADS = D_MODEL // D_HEAD
D_FF = 3 * D_MODEL
EPS = 1e-6


def _rms_norm(x, scale):
    ms = np.mean(x * x, axis=-1, keepdims=True)
    return x * (scale / np.sqrt(ms + EPS))


def _apply_rot(x, cos, sin):
    # x: [n, nh, l, dh]; cos/sin: [n, nh, l, dh//2]
    h = x.shape[-1] // 2
    x1, x2 = x[..., :h], x[..., h:]
    return np.concatenate([x1 * cos - x2 * sin, x2 * cos + x1 * sin], axis=-1)


def _softmax(s):
    m = np.max(s, axis=-1, keepdims=True)
    e = np.exp(s - m)
    return e / np.sum(e, axis=-1, keepdims=True)


def _forward(x, pos, norm1_scale, qkv_w, attn_scale, freqs, out_w,
             norm2_scale, up_w, down_w):
    n, l, d = x.shape
    skip = x
    h = _rms_norm(x, norm1_scale)
    qkv = h.reshape(n * l, d) @ qkv_w.T
    qkv = qkv.reshape(n, l, 3, N_HEADS, D_HEAD)
    qkv = np.transpose(qkv, (2, 0, 3, 1, 4))  # [3, n, nh, l, dh]
    q, k, v = qkv[0], qkv[1], qkv[2]
    sc = attn_scale[:, None, None]
    sqrt_sc = np.sqrt(sc)
    q = q * (sqrt_sc / np.sqrt(np.sum(q * q, -1, keepdims=True) + EPS))
    k = k * (sqrt_sc / np.sqrt(np.sum(k * k, -1, keepdims=True) + EPS))
    theta_h = pos[..., None, 0:1] * freqs  # [n, l, nh, dh//4]
    theta_w = pos[..., None, 1:2] * freqs
    theta = np.concatenate([theta_h, theta_w], axis=-1)  # [n, l, nh, dh//2]
    theta = np.moveaxis(theta, -2, -3)  # [n, nh, l, dh//2]
    cos, sin = np.cos(theta), np.sin(theta)
    q = _apply_rot(q, cos, sin)
    k = _apply_rot(k, cos, sin)
    o = np.empty((n, N_HEADS, l, D_HEAD), dtype=np.float32)
    for b in range(n):
        for hd in range(N_HEADS):
            s = q[b, hd] @ k[b, hd].T
            a = _softmax(s)
            o[b, hd] = a @ v[b, hd]
    o = np.transpose(o, (0, 2, 1, 3)).reshape(n, l, d)
    x = o.reshape(n * l, d) @ out_w.T
    x = x.reshape(n, l, d) + skip
    skip = x
    h = _rms_norm(x, norm2_scale)
    u = h.reshape(n * l, d) @ up_w.T
    a, g = u[:, :D_FF], u[:, D_FF:]
    hf = a * (g / (1.0 + np.exp(-g)))
    y = hf @ down_w.T
    return y.reshape(n, l, d) + skip


def kernel(**inputs):
    args = {k: np.asarray(v, dtype=np.float32) for k, v in inputs.items()}
    out = _forward(
        args["x"], args["pos"], args["norm1_scale"], args["qkv_w"],
        args["attn_scale"], args["freqs"], args["out_w"],
        args["norm2_scale"], args["up_w"], args["down_w"],
    )
    return out.astype(np.float32)


# revision 2
# speedup vs baseline: 2.4587x; 2.4587x over previous
import math
import numpy as np

D_MODEL = 1024
D_HEAD = 64
N_HEADS = D_MODEL // D_HEAD
D_FF = 3 * D_MODEL
EPS = 1e-6


def _rms_norm(x, scale):
    ms = np.mean(x * x, axis=-1, keepdims=True)
    return x * (scale / np.sqrt(ms + EPS))


def _apply_rot(x, cos, sin):
    # x: [n, nh, l, dh]; cos/sin: [n, nh, l, dh//2]
    h = x.shape[-1] // 2
    x1, x2 = x[..., :h], x[..., h:]
    return np.concatenate([x1 * cos - x2 * sin, x2 * cos + x1 * sin], axis=-1)


def _softmax(s):
    m = np.max(s, axis=-1, keepdims=True)
    e = np.exp(s - m)
    return e / np.sum(e, axis=-1, keepdims=True)


def _forward(x, pos, norm1_scale, qkv_w, attn_scale, freqs, out_w,
             norm2_scale, up_w, down_w):
    n, l, d = x.shape
    skip = x
    h = _rms_norm(x, norm1_scale)
    qkv = h.reshape(n * l, d) @ qkv_w.T
    qkv = qkv.reshape(n, l, 3, N_HEADS, D_HEAD)
    qkv = np.transpose(qkv, (2, 0, 3, 1, 4))  # [3, n, nh, l, dh]
    q, k, v = qkv[0], qkv[1], qkv[2]
    sc = attn_scale[:, None, None]
    sqrt_sc = np.sqrt(sc)
    q = q * (sqrt_sc / np.sqrt(np.sum(q * q, -1, keepdims=True) + EPS))
    k = k * (sqrt_sc / np.sqrt(np.sum(k * k, -1, keepdims=True) + EPS))
    theta_h = pos[..., None, 0:1] * freqs  # [n, l, nh, dh//4]
    theta_w = pos[..., None, 1:2] * freqs
    theta = np.concatenate([theta_h, theta_w], axis=-1)  # [n, l, nh, dh//2]
    theta = np.moveaxis(theta, -2, -3)  # [n, nh, l, dh//2]
    cos, sin = np.cos(theta), np.sin(theta)
    q = _apply_rot(q, cos, sin)
    k = _apply_rot(k, cos, sin)
    o = np.empty((n, N_HEADS, l, D_HEAD), dtype=np.float32)
    for b in range(n):
        for hd in range(N_HEADS):
            s = q[b, hd] @ k[b, hd].T
            a = _softmax(s)
            o[b, hd] = a @ v[b, hd]
    o = np.transpose(o, (0, 2, 1, 3)).reshape(n, l, d)
    x = o.reshape(n * l, d) @ out_w.T
    x = x.reshape(n, l, d) + skip
    skip = x
    h = _rms_norm(x, norm2_scale)
    u = h.reshape(n * l, d) @ up_w.T
    a, g = u[:, :D_FF], u[:, D_FF:]
    hf = a * (g / (1.0 + np.exp(-g)))
    y = hf @ down_w.T
    return y.reshape(n, l, d) + skip


def _forward_jax(args):
    import jax
    import jax.numpy as jnp

    def fwd(x, pos, norm1_scale, qkv_w, attn_scale, freqs, out_w,
            norm2_scale, up_w, down_w):
        def rms(t, scale):
            ms = jnp.mean(t * t, axis=-1, keepdims=True)
            return t * (scale * jax.lax.rsqrt(ms + EPS))

        def rot(t, cos, sin):
            t1, t2 = jnp.split(t, 2, axis=-1)
            return jnp.concatenate([t1 * cos - t2 * sin, t2 * cos + t1 * sin], -1)

        n, l, d = x.shape
        skip = x
        h = rms(x, norm1_scale)
        qkv = h @ qkv_w.T
        qkv = qkv.reshape(n, l, 3, N_HEADS, D_HEAD)
        qkv = jnp.transpose(qkv, (2, 0, 3, 1, 4))
        q, k, v = qkv[0], qkv[1], qkv[2]
        sc = attn_scale[:, None, None]
        sqrt_sc = jnp.sqrt(sc)
        q = q * (sqrt_sc * jax.lax.rsqrt(jnp.sum(q * q, -1, keepdims=True) + EPS))
        k = k * (sqrt_sc * jax.lax.rsqrt(jnp.sum(k * k, -1, keepdims=True) + EPS))
        theta_h = pos[..., None, 0:1] * freqs
        theta_w = pos[..., None, 1:2] * freqs
        theta = jnp.concatenate([theta_h, theta_w], axis=-1)
        theta = jnp.moveaxis(theta, -2, -3)
        cos, sin = jnp.cos(theta), jnp.sin(theta)
        q = rot(q, cos, sin)
        k = rot(k, cos, sin)
        scores = jnp.einsum('nhqd,nhkd->nhqk', q, k)
        attn = jax.nn.softmax(scores, axis=-1)
        o = jnp.einsum('nhqk,nhkd->nhqd', attn, v)
        o = jnp.transpose(o, (0, 2, 1, 3)).reshape(n, l, d)
        x2 = o @ out_w.T + skip
        skip2 = x2
        h2 = rms(x2, norm2_scale)
        u = h2 @ up_w.T
        a, g = jnp.split(u, 2, axis=-1)
        h3 = a * jax.nn.silu(g)
        return h3 @ down_w.T + skip2

    f = jax.jit(fwd)
    out = f(args["x"], args["pos"], args["norm1_scale"], args["qkv_w"],
            args["attn_scale"], args["freqs"], args["out_w"],
            args["norm2_scale"], args["up_w"], args["down_w"])
    return np.asarray(out, dtype=np.float32)


def kernel(**inputs):
    args = {k: np.asarray(v, dtype=np.float32) for k, v in inputs.items()}
    try:
        return _forward_jax(args)
    except Exception:
        out = _forward(
            args["x"], args["pos"], args["norm1_scale"], args["qkv_w"],
            args["attn_scale"], args["freqs"], args["out_w"],
            args["norm2_scale"], args["up_w"], args["down_w"],
        )
        return out.astype(np.float32)


# revision 4
# speedup vs baseline: 2.9974x; 1.2191x over previous
import math
import numpy as np

D_MODEL = 1024
D_HEAD = 64
N_HEADS = D_MODEL // D_HEAD
D_FF = 3 * D_MODEL
EPS = 1e-6


def _rms_norm(x, scale):
    ms = np.mean(x * x, axis=-1, keepdims=True)
    return x * (scale / np.sqrt(ms + EPS))


def _apply_rot(x, cos, sin):
    # x: [n, nh, l, dh]; cos/sin: [n, nh, l, dh//2]
    h = x.shape[-1] // 2
    x1, x2 = x[..., :h], x[..., h:]
    return np.concatenate([x1 * cos - x2 * sin, x2 * cos + x1 * sin], axis=-1)


def _softmax(s):
    m = np.max(s, axis=-1, keepdims=True)
    e = np.exp(s - m)
    return e / np.sum(e, axis=-1, keepdims=True)


def _forward(x, pos, norm1_scale, qkv_w, attn_scale, freqs, out_w,
             norm2_scale, up_w, down_w):
    n, l, d = x.shape
    skip = x
    h = _rms_norm(x, norm1_scale)
    qkv = h.reshape(n * l, d) @ qkv_w.T
    qkv = qkv.reshape(n, l, 3, N_HEADS, D_HEAD)
    qkv = np.transpose(qkv, (2, 0, 3, 1, 4))  # [3, n, nh, l, dh]
    q, k, v = qkv[0], qkv[1], qkv[2]
    sc = attn_scale[:, None, None]
    sqrt_sc = np.sqrt(sc)
    q = q * (sqrt_sc / np.sqrt(np.sum(q * q, -1, keepdims=True) + EPS))
    k = k * (sqrt_sc / np.sqrt(np.sum(k * k, -1, keepdims=True) + EPS))
    theta_h = pos[..., None, 0:1] * freqs  # [n, l, nh, dh//4]
    theta_w = pos[..., None, 1:2] * freqs
    theta = np.concatenate([theta_h, theta_w], axis=-1)  # [n, l, nh, dh//2]
    theta = np.moveaxis(theta, -2, -3)  # [n, nh, l, dh//2]
    cos, sin = np.cos(theta), np.sin(theta)
    q = _apply_rot(q, cos, sin)
    k = _apply_rot(k, cos, sin)
    o = np.empty((n, N_HEADS, l, D_HEAD), dtype=np.float32)
    for b in range(n):
        for hd in range(N_HEADS):
            s = q[b, hd] @ k[b, hd].T
            a = _softmax(s)
            o[b, hd] = a @ v[b, hd]
    o = np.transpose(o, (0, 2, 1, 3)).reshape(n, l, d)
    x = o.reshape(n * l, d) @ out_w.T
    x = x.reshape(n, l, d) + skip
    skip = x
    h = _rms_norm(x, norm2_scale)
    u = h.reshape(n * l, d) @ up_w.T
    a, g = u[:, :D_FF], u[:, D_FF:]
    hf = a * (g / (1.0 + np.exp(-g)))
    y = hf @ down_w.T
    return y.reshape(n, l, d) + skip


def _forward_jax(args):
    import jax
    import jax.numpy as jnp

    bf = jnp.bfloat16
    f32 = jnp.float32

    def mm(a, b):
        return jnp.matmul(a.astype(bf), b.astype(bf),
                          preferred_element_type=f32)

    def fwd(x, pos, norm1_scale, qkv_w, attn_scale, freqs, out_w,
            norm2_scale, up_w, down_w):
        def rms(t, scale):
            ms = jnp.mean(t * t, axis=-1, keepdims=True)
            return t * (scale * jax.lax.rsqrt(ms + EPS))

        def rot(t, cos, sin):
            t1, t2 = jnp.split(t, 2, axis=-1)
            return jnp.concatenate([t1 * cos - t2 * sin, t2 * cos + t1 * sin], -1)

        n, l, d = x.shape
        skip = x
        h = rms(x, norm1_scale)
        qkv = mm(h, qkv_w.T)
        qkv = qkv.reshape(n, l, 3, N_HEADS, D_HEAD)
        qkv = jnp.transpose(qkv, (2, 0, 3, 1, 4))
        q, k, v = qkv[0], qkv[1], qkv[2]
        sc = attn_scale[:, None, None]
        sqrt_sc = jnp.sqrt(sc)
        q = q * (sqrt_sc * jax.lax.rsqrt(jnp.sum(q * q, -1, keepdims=True) + EPS))
        k = k * (sqrt_sc * jax.lax.rsqrt(jnp.sum(k * k, -1, keepdims=True) + EPS))
        theta_h = pos[..., None, 0:1] * freqs
        theta_w = pos[..., None, 1:2] * freqs
        theta = jnp.concatenate([theta_h, theta_w], axis=-1)
        theta = jnp.moveaxis(theta, -2, -3)
        cos, sin = jnp.cos(theta), jnp.sin(theta)
        q = rot(q, cos, sin)
        k = rot(k, cos, sin)
        scores = jnp.einsum('nhqd,nhkd->nhqk', q.astype(bf), k.astype(bf),
                            preferred_element_type=f32)
        attn = jax.nn.softmax(scores, axis=-1)
        o = jnp.einsum('nhqk,nhkd->nhqd', attn.astype(bf), v.astype(bf),
                       preferred_element_type=f32)
        o = jnp.transpose(o, (0, 2, 1, 3)).reshape(n, l, d)
        x2 = mm(o, out_w.T) + skip
        skip2 = x2
        h2 = rms(x2, norm2_scale)
        u = mm(h2, up_w.T)
        a, g = jnp.split(u, 2, axis=-1)
        h3 = a * jax.nn.silu(g)
        return mm(h3, down_w.T) + skip2

    f = jax.jit(fwd)
    out = f(args["x"], args["pos"], args["norm1_scale"], args["qkv_w"],
            args["attn_scale"], args["freqs"], args["out_w"],
            args["norm2_scale"], args["up_w"], args["down_w"])
    return np.asarray(out, dtype=np.float32)


def kernel(**inputs):
    args = {k: np.asarray(v, dtype=np.float32) for k, v in inputs.items()}
    try:
        return _forward_jax(args)
    except Exception:
        out = _forward(
            args["x"], args["pos"], args["norm1_scale"], args["qkv_w"],
            args["attn_scale"], args["freqs"], args["out_w"],
            args["norm2_scale"], args["up_w"], args["down_w"],
        )
        return out.astype(np.float32)
